# revision 14
# baseline (speedup 1.0000x reference)
"""Trainium2 Bass kernel for DimensionalAttentionMask.

Computes, for token_ids (B=4, T=4096), dim_embedding (50257, 8),
compatibility (8, 8):

    probs = softmax(dim_embedding[token_ids], axis=-1)        # (B,T,8)
    compat = einsum('btc,cd,bsd->bts', probs, C, probs)       # (B,T,T)
    out = sigmoid(compat)*2 - 1  ==  tanh(compat / 2)         # (B,T,T)

Sharding: 8 cores, each computes a (2048, 4096) block of query rows:
core k -> batch k//2, query rows [(k%2)*2048, (k%2)*2048+2048).

Per-core device program:
  1. indirect-DMA gather of 6144 embedding rows (4096 keys + 2048
     queries for this core) into SBUF, token r at partition r%128.
  2. softmax over the 8 categories (exp, grouped reduce, reciprocal, mul).
  3. PE transposes (128,8) -> (8,128) to build pT (8, 6144) with
     categories on partitions.
  4. qT = compatibility^T @ pT[:, query part]  (8, 2048).
  5. 16x8 tiles: PSUM(128,512) = qT_m^T @ pT_n; ACT computes
     tanh(0.5*x) PSUM->SBUF; 2 MiB contiguous row-stripe DMA to DRAM.
"""

import numpy as np

B, T = 4, 4096
VOCAB, C = 50257, 8
HALF = 32767             # int16 index ceiling for dma_gather; vocab is split
PAD = 64                 # embedding rows padded to 64 f32 = 256 B for dma_gather
NCORES = 8
TQ = T // 2              # query rows per core
GK = T // 128            # 32 key groups of 128 tokens
GQ = TQ // 128           # 16 query groups
G = GK + GQ              # 48 gathered groups per core
NTILE = 512              # key columns per matmul (one PSUM bank, fp32)
# output stripe schedule, in m-tiles (128 rows = 2 MiB each); uniform 2 MiB
# keeps stripe-slot reuse faster than the DMA drain for this PE-bound kernel
STRIPE_M = [1] * 16

_CACHE = {}
LAST_RESULT = None       # BassKernelResults of the most recent device run


def _build():
    from contextlib import ExitStack

    import concourse.bass as bass
    import concourse.mybir as mybir
    import concourse.tile as tile
    from concourse import bacc
    from concourse.masks import make_identity

    dt = mybir.dt
    # Bacc (not Bass): its finalize() runs move_matmul_waits_to_ldweights +
    # generate_event_semaphores, which split multi-sem waits that walrus's
    # matmul codegen (1 wait slot) rejects.
    nc = bacc.Bacc(
        "TRN2", target_bir_lowering=False, debug=False, num_devices=NCORES
    )

    slabs = nc.declare_dram_parameter(
        "slabs", [2, HALF + 1, PAD], dt.float32, isOutput=False
    )
    comp = nc.declare_dram_parameter("comp", [C, C], dt.float32, isOutput=False)
    idx1 = nc.declare_dram_parameter("idx1", [128, G * 8], dt.int16, isOutput=False)
    idx2 = nc.declare_dram_parameter("idx2", [128, G * 8], dt.int16, isOutput=False)
    out = nc.declare_dram_parameter("out", [TQ, T], dt.float32, isOutput=True)

    with tile.TileContext(nc) as tc, ExitStack() as ctx:
        sb = ctx.enter_context(tc.tile_pool(name="sb", bufs=1))
        ps = ctx.enter_context(tc.tile_pool(name="ps", bufs=8, space="PSUM"))
        stripes = ctx.enter_context(tc.tile_pool(name="stripe", bufs=3))

        idx1_t = sb.tile([128, G * 8], dt.int16)
        nc.sync.dma_start(idx1_t[:], idx1[:])
        idx2_t = sb.tile([128, G * 8], dt.int16)
        nc.sync.dma_start(idx2_t[:], idx2[:])
        comp_t = sb.tile([C, C], dt.float32)
        nc.sync.dma_start(comp_t[:], comp[:])
        # PE matmuls tolerate only one sync-wait in walrus codegen, so
        # every SBUF operand PE reads is last touched by DVE: copy the
        # gpsimd-built identity and the DMA-loaded compatibility via DVE.
        ident0 = sb.tile([128, 128], dt.float32)
        make_identity(nc, ident0[:])
        ident = sb.tile([128, 128], dt.float32)
        nc.vector.tensor_copy(ident[:], ident0[:])
        compv = sb.tile([C, C], dt.float32)
        nc.vector.tensor_copy(compv[:], comp_t[:])

        # Embedding gather via dma_gather (int16 idxs, 256B rows). The
        # vocab exceeds int16, so it is split into two slabs with a zero
        # row at 0; each token hits its row in one slab and row 0 in the
        # other, and the two gathers are summed. Token j = g*128+p lands
        # at out[p, g, :]. single_packet=True faults the Q7 above ~512
        # idxs (HW-bisected), so keep it off.
        g1 = sb.tile([128, G, PAD], dt.float32)
        g2 = sb.tile([128, G, PAD], dt.float32)
        nc.gpsimd.dma_gather(
            out_ap=g1[:], in_ap=slabs[0], idxs_ap=idx1_t[:],
            num_idxs=G * 128, num_idxs_reg=G * 128, elem_size=PAD,
            single_packet=False,
        )
        nc.gpsimd.dma_gather(
            out_ap=g2[:], in_ap=slabs[1], idxs_ap=idx2_t[:],
            num_idxs=G * 128, num_idxs_reg=G * 128, elem_size=PAD,
            single_packet=False,
        )
        gth = sb.tile([128, G, C], dt.float32)
        nc.vector.tensor_add(gth[:], g1[:, :, 0:C], g2[:, :, 0:C])

        # softmax over the 8 categories of each token
        ex = sb.tile([128, G, C], dt.float32)
        nc.scalar.activation(ex[:], gth[:], mybir.ActivationFunctionType.Exp)
        ssum = sb.tile([128, G], dt.float32)
        nc.vector.reduce_sum(out=ssum[:], in_=ex[:], axis=mybir.AxisListType.X)
        rsum = sb.tile([128, G], dt.float32)
        nc.vector.reciprocal(rsum[:], ssum[:])
        probs = sb.tile([128, G, C], dt.float32)
        nc.vector.tensor_mul(
            probs[:],
            ex[:],
            rsum[:].unsqueeze(2).to_broadcast([128, G, C]),
        )

        # pT[c, g*128 + p] = probs[p, g, c]; groups 0..31 keys, 32..47
        # queries. Query groups transpose FIRST so the qT projection (a
        # dependency of every main matmul) is ready before the key waves.
        pT = sb.tile([C, G * 128], dt.float32)
        for j in list(range(GK // 4, G // 4)) + list(range(GK // 4)):
            tp = ps.tile([C, 512], dt.float32, tag="ps", name=f"tp{j}")
            for i in range(4):
                g = j * 4 + i
                nc.tensor.transpose(
                    out=tp[:, i * 128 : (i + 1) * 128],
                    in_=probs[:, g, :],
                    identity=ident[:],
                )
            nc.vector.tensor_copy(pT[:, j * 512 : (j + 1) * 512], tp[:])

        # qT = compatibility^T @ pT[:, query part]   (8, 2048)
        qT = sb.tile([C, TQ], dt.float32)
        for i in range(TQ // NTILE):
            qp = ps.tile([C, NTILE], dt.float32, tag="ps", name=f"qp{i}")
            nc.tensor.matmul(
                out=qp[:],
                lhsT=compv[:],
                rhs=pT[:, GK * 128 + i * NTILE : GK * 128 + (i + 1) * NTILE],
                start=True,
                stop=True,
            )
            nc.vector.tensor_copy(qT[:, i * NTILE : (i + 1) * NTILE], qp[:])

        # main: compat tile = qT_m^T @ pT_n, tanh(x/2) over matmul PAIRS
        # (half the ACT dispatches), variable-size contiguous stripe DMAs
        m = 0
        for nm in STRIPE_M:
            stripe = stripes.tile([128, nm * T], dt.float32, name="stripe")
            for s_ in range(nm):
                msl = slice((m + s_) * 128, (m + s_ + 1) * 128)
                for n in range(T // NTILE):
                    po = ps.tile(
                        [128, NTILE], dt.float32, tag="ps",
                        name=f"po{m + s_}_{n}",
                    )
                    nc.tensor.matmul(
                        out=po[:],
                        lhsT=qT[:, msl],
                        rhs=pT[:, n * NTILE : (n + 1) * NTILE],
                        start=True,
                        stop=True,
                    )
                    nc.scalar.activation(
                        stripe[:, s_ * T + n * NTILE : s_ * T + (n + 1) * NTILE],
                        po[:],
                        mybir.ActivationFunctionType.Tanh,
                        scale=0.5,
                    )
            dst = out[m * 128 : (m + nm) * 128, :]
            if nm > 1:
                dst = dst.rearrange("(s p) c -> p (s c)", s=nm)
            nc.sync.dma_start(dst, stripe[:])
            m += nm

    return nc


def _get_nc():
    if "nc" not in _CACHE:
        nc = _build()
        # Bacc defers register allocation to finalize(); the bass2jax SPMD
        # path serializes nc.m as-is, so finalize before handing it over.
        nc.finalize()
        _CACHE["nc"] = nc
    return _CACHE["nc"]


def _make_idx(tok_b: np.ndarray, t0: int):
    """int16 index pair for the two-slab gather, wrapped for dma_gather:
    logical token j (= g*128+p) sits at idx[j%16, j//16], replicated to
    all 8 GPSIMD-core partition groups."""
    tokens = np.concatenate([tok_b, tok_b[t0 : t0 + TQ]]).astype(np.int64)
    w = tokens.reshape(G * 8, 16).T  # w[p, s] = tokens[s*16+p]
    i1 = np.where(w < HALF, w + 1, 0).astype(np.int16)
    i2 = np.where(w >= HALF, w - HALF + 1, 0).astype(np.int16)
    return np.tile(i1, (8, 1)), np.tile(i2, (8, 1))


def _make_slabs(emb: np.ndarray) -> np.ndarray:
    slabs = np.zeros((2, HALF + 1, PAD), np.float32)
    slabs[0, 1 : HALF + 1, :C] = emb[0:HALF]
    slabs[1, 1 : VOCAB - HALF + 1, :C] = emb[HALF:]
    return slabs


def _make_in_maps(tok, emb, comp):
    slabs = _make_slabs(emb)
    in_maps = []
    for k in range(NCORES):
        b, t0 = k // 2, (k % 2) * TQ
        i1, i2 = _make_idx(tok[b], t0)
        in_maps.append({"slabs": slabs, "comp": comp, "idx1": i1, "idx2": i2})
    return in_maps


def kernel(token_ids, dim_embedding, compatibility):
    global LAST_RESULT
    from concourse.bass_utils import run_bass_kernel_spmd

    tok = np.asarray(token_ids)
    emb = np.ascontiguousarray(np.asarray(dim_embedding, dtype=np.float32))
    comp = np.ascontiguousarray(np.asarray(compatibility, dtype=np.float32))
    assert tok.shape == (B, T) and emb.shape == (VOCAB, C) and comp.shape == (C, C)

    nc = _get_nc()
    in_maps = _make_in_maps(tok, emb, comp)

    res = run_bass_kernel_spmd(nc, in_maps, list(range(NCORES)))
    LAST_RESULT = res

    full = np.empty((B, T, T), dtype=np.float32)
    for k in range(NCORES):
        b, t0 = k // 2, (k % 2) * TQ
        full[b, t0 : t0 + TQ, :] = res.results[k]["out"]
    return full


# revision 15
# speedup vs baseline: 1.0532x; 1.0532x over previous
"""Trainium2 Bass kernel for DimensionalAttentionMask.

Computes, for token_ids (B=4, T=4096), dim_embedding (50257, 8),
compatibility (8, 8):

    probs = softmax(dim_embedding[token_ids], axis=-1)        # (B,T,8)
    compat = einsum('btc,cd,bsd->bts', probs, C, probs)       # (B,T,T)
    out = sigmoid(compat)*2 - 1  ==  tanh(compat / 2)         # (B,T,T)

Sharding: 8 cores, each computes a (2048, 4096) block of query rows:
core k -> batch k//2, query rows [(k%2)*2048, (k%2)*2048+2048).

Per-core device program:
  1. dma_gather of 6144 embedding rows (4096 keys + 2048 queries for
     this core, 256 B-padded) from two int16-indexed table slabs,
     summed; token j = g*128+p lands at SBUF slot (p, g).
  2. softmax over the 8 categories (exp, grouped reduce, reciprocal, mul).
  3. PE transposes (128,8) -> (8,128) to build pT (8, 6144) with
     categories on partitions; query groups first so qT is ready early.
  4. qT = compatibility^T @ pT[:, query part]  (8, 2048).
  5. 16x8 tiles: PSUM(128,512) = qT_m^T @ pT_n (fp32 matmul, K=8);
     ACT computes tanh(0.5*x) PSUM->SBUF; 16 x 2 MiB contiguous
     row-stripe DMAs to DRAM overlap the whole main loop.
"""

import numpy as np

B, T = 4, 4096
VOCAB, C = 50257, 8
HALF = 32767             # int16 index ceiling for dma_gather; vocab is split
PAD = 64                 # embedding rows padded to 64 f32 = 256 B for dma_gather
NCORES = 8
TQ = T // 2              # query rows per core
GK = T // 128            # 32 key groups of 128 tokens
GQ = TQ // 128           # 16 query groups
G = GK + GQ              # 48 gathered groups per core
NTILE = 512              # key columns per matmul (one PSUM bank, fp32)
# output stripe schedule, in m-tiles (128 rows = 2 MiB each); uniform 2 MiB
# keeps stripe-slot reuse faster than the DMA drain for this PE-bound kernel
STRIPE_M = [1] * 16

_CACHE = {}
LAST_RESULT = None       # BassKernelResults of the most recent device run


def _build():
    from contextlib import ExitStack

    import concourse.bass as bass
    import concourse.mybir as mybir
    import concourse.tile as tile
    from concourse import bacc
    from concourse.masks import make_identity

    dt = mybir.dt
    # Bacc (not Bass): its finalize() runs move_matmul_waits_to_ldweights +
    # generate_event_semaphores, which split multi-sem waits that walrus's
    # matmul codegen (1 wait slot) rejects.
    nc = bacc.Bacc(
        "TRN2", target_bir_lowering=False, debug=False, num_devices=NCORES
    )

    slabs = nc.declare_dram_parameter(
        "slabs", [2, HALF + 1, PAD], dt.float32, isOutput=False
    )
    comp = nc.declare_dram_parameter("comp", [C, C], dt.float32, isOutput=False)
    idx1 = nc.declare_dram_parameter("idx1", [128, G * 8], dt.int16, isOutput=False)
    idx2 = nc.declare_dram_parameter("idx2", [128, G * 8], dt.int16, isOutput=False)
    out = nc.declare_dram_parameter("out", [TQ, T], dt.float32, isOutput=True)

    with tile.TileContext(nc) as tc, ExitStack() as ctx:
        sb = ctx.enter_context(tc.tile_pool(name="sb", bufs=1))
        ps = ctx.enter_context(tc.tile_pool(name="ps", bufs=8, space="PSUM"))
        stripes = ctx.enter_context(tc.tile_pool(name="stripe", bufs=3))

        idx1_t = sb.tile([128, G * 8], dt.int16)
        nc.sync.dma_start(idx1_t[:], idx1[:])
        idx2_t = sb.tile([128, G * 8], dt.int16)
        nc.sync.dma_start(idx2_t[:], idx2[:])
        comp_t = sb.tile([C, C], dt.float32)
        nc.sync.dma_start(comp_t[:], comp[:])
        # PE matmuls tolerate only one sync-wait in walrus codegen, so
        # every SBUF operand PE reads is last touched by DVE: copy the
        # gpsimd-built identity and the DMA-loaded compatibility via DVE.
        ident0 = sb.tile([128, 128], dt.float32)
        make_identity(nc, ident0[:])
        ident = sb.tile([128, 128], dt.float32)
        nc.vector.tensor_copy(ident[:], ident0[:])
        compv = sb.tile([C, C], dt.float32)
        nc.vector.tensor_copy(compv[:], comp_t[:])

        # Embedding gather via dma_gather (int16 idxs, 256B rows). The
        # vocab exceeds int16, so it is split into two slabs with a zero
        # row at 0; each token hits its row in one slab and row 0 in the
        # other, and the two gathers are summed. Token j = g*128+p lands
        # at out[p, g, :]. single_packet=True faults the Q7 above ~512
        # idxs (HW-bisected), so keep it off.
        g1 = sb.tile([128, G, PAD], dt.float32)
        g2 = sb.tile([128, G, PAD], dt.float32)
        nc.gpsimd.dma_gather(
            out_ap=g1[:], in_ap=slabs[0], idxs_ap=idx1_t[:],
            num_idxs=G * 128, num_idxs_reg=G * 128, elem_size=PAD,
            single_packet=False,
        )
        nc.gpsimd.dma_gather(
            out_ap=g2[:], in_ap=slabs[1], idxs_ap=idx2_t[:],
            num_idxs=G * 128, num_idxs_reg=G * 128, elem_size=PAD,
            single_packet=False,
        )
        gth = sb.tile([128, G, C], dt.float32)
        nc.vector.tensor_add(gth[:], g1[:, :, 0:C], g2[:, :, 0:C])

        # softmax over the 8 categories of each token
        ex = sb.tile([128, G, C], dt.float32)
        nc.scalar.activation(ex[:], gth[:], mybir.ActivationFunctionType.Exp)
        ssum = sb.tile([128, G], dt.float32)
        nc.vector.reduce_sum(out=ssum[:], in_=ex[:], axis=mybir.AxisListType.X)
        rsum = sb.tile([128, G], dt.float32)
        nc.vector.reciprocal(rsum[:], ssum[:])
        probs = sb.tile([128, G, C], dt.float32)
        nc.vector.tensor_mul(
            probs[:],
            ex[:],
            rsum[:].unsqueeze(2).to_broadcast([128, G, C]),
        )

        # pT[c, g*128 + p] = probs[p, g, c]; groups 0..31 keys, 32..47
        # queries. Query groups transpose FIRST so the qT projection (a
        # dependency of every main matmul) is ready before the key waves.
        pT = sb.tile([C, G * 128], dt.float32)
        for j in list(range(GK // 4, G // 4)) + list(range(GK // 4)):
            tp = ps.tile([C, 512], dt.float32, tag="ps", name=f"tp{j}")
            for i in range(4):
                g = j * 4 + i
                nc.tensor.transpose(
                    out=tp[:, i * 128 : (i + 1) * 128],
                    in_=probs[:, g, :],
                    identity=ident[:],
                )
            nc.vector.tensor_copy(pT[:, j * 512 : (j + 1) * 512], tp[:])

        # qT = compatibility^T @ pT[:, query part]   (8, 2048)
        qT = sb.tile([C, TQ], dt.float32)
        for i in range(TQ // NTILE):
            qp = ps.tile([C, NTILE], dt.float32, tag="ps", name=f"qp{i}")
            nc.tensor.matmul(
                out=qp[:],
                lhsT=compv[:],
                rhs=pT[:, GK * 128 + i * NTILE : GK * 128 + (i + 1) * NTILE],
                start=True,
                stop=True,
            )
            nc.vector.tensor_copy(qT[:, i * NTILE : (i + 1) * NTILE], qp[:])

        # main: compat tile = qT_m^T @ pT_n, tanh(x/2) over matmul PAIRS
        # (half the ACT dispatches), variable-size contiguous stripe DMAs
        m = 0
        for nm in STRIPE_M:
            stripe = stripes.tile([128, nm * T], dt.float32, name="stripe")
            for s_ in range(nm):
                msl = slice((m + s_) * 128, (m + s_ + 1) * 128)
                for n in range(T // NTILE):
                    po = ps.tile(
                        [128, NTILE], dt.float32, tag="ps",
                        name=f"po{m + s_}_{n}",
                    )
                    nc.tensor.matmul(
                        out=po[:],
                        lhsT=qT[:, msl],
                        rhs=pT[:, n * NTILE : (n + 1) * NTILE],
                        start=True,
                        stop=True,
                    )
                    nc.scalar.activation(
                        stripe[:, s_ * T + n * NTILE : s_ * T + (n + 1) * NTILE],
                        po[:],
                        mybir.ActivationFunctionType.Tanh,
                        scale=0.5,
                    )
            dst = out[m * 128 : (m + nm) * 128, :]
            if nm > 1:
                dst = dst.rearrange("(s p) c -> p (s c)", s=nm)
            nc.sync.dma_start(dst, stripe[:])
            m += nm

    return nc


def _get_nc():
    if "nc" not in _CACHE:
        nc = _build()
        # Bacc defers register allocation to finalize(); the bass2jax SPMD
        # path serializes nc.m as-is, so finalize before handing it over.
        nc.finalize()
        _CACHE["nc"] = nc
    return _CACHE["nc"]


def _make_idx(tok_b: np.ndarray, t0: int):
    """int16 index pair for the two-slab gather, wrapped for dma_gather:
    logical token j (= g*128+p) sits at idx[j%16, j//16], replicated to
    all 8 GPSIMD-core partition groups."""
    tokens = np.concatenate([tok_b, tok_b[t0 : t0 + TQ]]).astype(np.int64)
    w = tokens.reshape(G * 8, 16).T  # w[p, s] = tokens[s*16+p]
    i1 = np.where(w < HALF, w + 1, 0).astype(np.int16)
    i2 = np.where(w >= HALF, w - HALF + 1, 0).astype(np.int16)
    return np.tile(i1, (8, 1)), np.tile(i2, (8, 1))


def _make_slabs(emb: np.ndarray) -> np.ndarray:
    slabs = np.zeros((2, HALF + 1, PAD), np.float32)
    slabs[0, 1 : HALF + 1, :C] = emb[0:HALF]
    slabs[1, 1 : VOCAB - HALF + 1, :C] = emb[HALF:]
    return slabs


def _make_in_maps(tok, emb, comp):
    slabs = _make_slabs(emb)
    in_maps = []
    for k in range(NCORES):
        b, t0 = k // 2, (k % 2) * TQ
        i1, i2 = _make_idx(tok[b], t0)
        in_maps.append({"slabs": slabs, "comp": comp, "idx1": i1, "idx2": i2})
    return in_maps


def kernel(token_ids, dim_embedding, compatibility):
    global LAST_RESULT
    from concourse.bass_utils import run_bass_kernel_spmd

    tok = np.asarray(token_ids)
    emb = np.ascontiguousarray(np.asarray(dim_embedding, dtype=np.float32))
    comp = np.ascontiguousarray(np.asarray(compatibility, dtype=np.float32))
    assert tok.shape == (B, T) and emb.shape == (VOCAB, C) and comp.shape == (C, C)

    nc = _get_nc()
    in_maps = _make_in_maps(tok, emb, comp)

    res = run_bass_kernel_spmd(nc, in_maps, list(range(NCORES)))
    LAST_RESULT = res

    full = np.empty((B, T, T), dtype=np.float32)
    for k in range(NCORES):
        b, t0 = k // 2, (k % 2) * TQ
        full[b, t0 : t0 + TQ, :] = res.results[k]["out"]
    return full


# revision 21
# speedup vs baseline: 1.0951x; 1.0397x over previous
"""Trainium2 Bass kernel for DimensionalAttentionMask.

Computes, for token_ids (B=4, T=4096), dim_embedding (50257, 8),
compatibility (8, 8):

    probs = softmax(dim_embedding[token_ids], axis=-1)        # (B,T,8)
    compat = einsum('btc,cd,bsd->bts', probs, C, probs)       # (B,T,T)
    out = sigmoid(compat)*2 - 1  ==  tanh(compat / 2)         # (B,T,T)

Sharding: 8 cores, each computes a (2048, 4096) block of query rows:
core k -> batch k//2, query rows [(k%2)*2048, (k%2)*2048+2048).

Per-core device program:
  1. dma_gather of the core's 4096 key rows (queries are a subset: the
     host orders key groups query-half-first and unshards the permuted
     output columns) from two int16-indexed 256 B-padded table slabs.
  2. softmax over the 8 categories; bf16 hi/lo split of the probs
     (hi = bf16 round, lo = bf16 round of the residual).
  3. hi and lo sit side by side on the free axis, so ONE (128,16) PE
     transpose per key group lands [ph; pl] contiguously at partitions
     0-15 (transpose cost is the 128 identity columns, independent of
     input width); an exact fp32 pTq is built for the query projection.
  4. qT = compatibility^T @ pTq (fp32), split hi/lo; per-chunk HWDGE
     SBUF-to-SBUF DMAs place the duplicated row groups, giving a
     CONTIGUOUS K=24 stack: lhsT=[qh;qh;ql] x rhs=[ph;pl;ph].
  5. ONE K=24 bf16 matmul per (128,512) tile computes qh*ph + qh*pl +
     ql*ph in a single pass (PE cycles depend only on N; dropped
     ql*pl ~ 2^-18); ACT computes tanh(0.5*x) over matmul PAIRS
     PSUM->SBUF; 16 x 2 MiB contiguous row-stripe DMAs overlap the
     whole main loop. Softmax runs in two group-chunks so transposes
     start while the second chunk is still in the softmax pipeline.
"""

import numpy as np

B, T = 4, 4096
VOCAB, C = 50257, 8
HALF = 32767             # int16 index ceiling for dma_gather; vocab is split
PAD = 64                 # embedding rows padded to 64 f32 = 256 B for dma_gather
NCORES = 8
TQ = T // 2              # query rows per core
GK = T // 128            # 32 key groups of 128 tokens
GQ = TQ // 128           # 16 query groups
# Each core's queries are a subset of its keys, so the host orders key
# groups QUERY-HALF-FIRST per core and the device gathers only the 4096
# keys; output columns come back in that order and the host unshards
# with two column-slice assignments.
G = GK                   # 32 gathered groups per core
NTILE = 512              # key columns per matmul (one PSUM bank)
KS = 24                  # contiguous stacked contraction: rows 0-7/8-15/16-23
                         # pair [qh;qh;ql] with [ph;pl;ph] (no gap rows)
# output stripe schedule, in m-tiles (128 rows = 2 MiB each); uniform 2 MiB
# keeps stripe-slot reuse faster than the DMA drain for this PE-bound kernel
STRIPE_M = [1] * 16

_CACHE = {}
LAST_RESULT = None       # BassKernelResults of the most recent device run


def _build():
    from contextlib import ExitStack

    import concourse.bass as bass
    import concourse.mybir as mybir
    import concourse.tile as tile
    from concourse import bacc
    from concourse.masks import make_identity

    dt = mybir.dt
    # Bacc (not Bass): its finalize() runs move_matmul_waits_to_ldweights +
    # generate_event_semaphores, which split multi-sem waits that walrus's
    # matmul codegen (1 wait slot) rejects.
    nc = bacc.Bacc(
        "TRN2", target_bir_lowering=False, debug=False, num_devices=NCORES
    )

    slabs = nc.declare_dram_parameter(
        "slabs", [2, HALF + 1, PAD], dt.float32, isOutput=False
    )
    comp = nc.declare_dram_parameter("comp", [C, C], dt.float32, isOutput=False)
    idx1 = nc.declare_dram_parameter("idx1", [128, G * 8], dt.int16, isOutput=False)
    idx2 = nc.declare_dram_parameter("idx2", [128, G * 8], dt.int16, isOutput=False)
    out = nc.declare_dram_parameter("out", [TQ, T], dt.float32, isOutput=True)

    with tile.TileContext(nc) as tc, ExitStack() as ctx:
        sb = ctx.enter_context(tc.tile_pool(name="sb", bufs=1))
        ps = ctx.enter_context(tc.tile_pool(name="ps", bufs=4, space="PSUM"))
        stripes = ctx.enter_context(tc.tile_pool(name="stripe", bufs=3))

        idx1_t = sb.tile([128, G * 8], dt.int16)
        nc.sync.dma_start(idx1_t[:], idx1[:])
        idx2_t = sb.tile([128, G * 8], dt.int16)
        nc.sync.dma_start(idx2_t[:], idx2[:])
        comp_t = sb.tile([C, C], dt.float32)
        nc.sync.dma_start(comp_t[:], comp[:])
        # PE matmuls tolerate only one sync-wait in walrus codegen, so
        # every SBUF operand PE reads is last touched by DVE: copy the
        # gpsimd-built identity and the DMA-loaded compatibility via DVE.
        ident0 = sb.tile([128, 128], dt.float32)
        make_identity(nc, ident0[:])
        ident = sb.tile([128, 128], dt.float32)
        nc.vector.tensor_copy(ident[:], ident0[:])
        identb = sb.tile([128, 128], dt.bfloat16)
        nc.vector.tensor_copy(identb[:], ident0[:])
        compv = sb.tile([C, C], dt.float32)
        nc.vector.tensor_copy(compv[:], comp_t[:])

        # Embedding gather via dma_gather (int16 idxs, 256B rows). The
        # vocab exceeds int16, so it is split into two slabs with a zero
        # row at 0; each token hits its row in one slab and row 0 in the
        # other, and the two gathers are summed. Token j = g*128+p lands
        # at out[p, g, :]. single_packet=True faults the Q7 above ~512
        # idxs (HW-bisected), so keep it off.
        g1 = sb.tile([128, G, PAD], dt.float32)
        g2 = sb.tile([128, G, PAD], dt.float32)
        nc.gpsimd.dma_gather(
            out_ap=g1[:], in_ap=slabs[0], idxs_ap=idx1_t[:],
            num_idxs=G * 128, num_idxs_reg=G * 128, elem_size=PAD,
            single_packet=False,
        )
        nc.gpsimd.dma_gather(
            out_ap=g2[:], in_ap=slabs[1], idxs_ap=idx2_t[:],
            num_idxs=G * 128, num_idxs_reg=G * 128, elem_size=PAD,
            single_packet=False,
        )
        gth = sb.tile([128, G, C], dt.float32)
        ex = sb.tile([128, G, C], dt.float32)
        ssum = sb.tile([128, G], dt.float32)
        rsum = sb.tile([128, G], dt.float32)
        probs = sb.tile([128, G, C], dt.float32)
        # softmax over the 8 categories, in two group-chunks so the
        # first transposes start while the second chunk is in flight
        H = G // 4
        for c0 in range(0, G, H):
            gs = slice(c0, c0 + H)
            nc.vector.tensor_add(gth[:, gs], g1[:, gs, 0:C], g2[:, gs, 0:C])
            nc.scalar.activation(
                ex[:, gs], gth[:, gs], mybir.ActivationFunctionType.Exp
            )
            nc.vector.reduce_sum(
                out=ssum[:, gs], in_=ex[:, gs], axis=mybir.AxisListType.X
            )
            nc.vector.reciprocal(rsum[:, gs], ssum[:, gs])
            nc.vector.tensor_mul(
                probs[:, gs],
                ex[:, gs],
                rsum[:, gs].unsqueeze(2).to_broadcast([128, H, C]),
            )

        # bf16 hi/lo split of probs: hi = bf16(p), lo = bf16(p - hi),
        # side by side on the free axis so one (128,16) transpose per
        # key group lands [ph; pl] contiguously at partitions 0-15. The
        # compensated product qh*ph + qh*pl + ql*ph (dropped ql*pl ~
        # 2^-18) then runs as ONE K=24 matmul per tile (matmul cycles
        # depend only on N, not K).
        probs_hl = sb.tile([128, G, 2 * C], dt.bfloat16)
        for c0 in range(0, G, H):
            gs = slice(c0, c0 + H)
            nc.vector.tensor_copy(probs_hl[:, gs, 0:C], probs[:, gs])
            nc.vector.tensor_sub(
                probs_hl[:, gs, C : 2 * C], probs[:, gs], probs_hl[:, gs, 0:C]
            )

        pTq = sb.tile([C, TQ], dt.float32)
        for j in range(GQ // 4):
            tq = ps.tile([C, 512], dt.float32, tag="ps", name=f"tq{j}")
            for i in range(4):
                g = j * 4 + i
                nc.tensor.transpose(
                    out=tq[:, i * 128 : (i + 1) * 128],
                    in_=probs[:, g, :],
                    identity=ident[:],
                )
            nc.vector.tensor_copy(pTq[:, j * 512 : (j + 1) * 512], tq[:])

        # pT_stack rows: [0:8]=ph, [8:16]=pl, [16:24]=ph (keys, bf16)
        pT_stack = sb.tile([KS, GK * 128], dt.bfloat16)
        for j in range(GK // 8):
            tp = ps.tile([KS, 1024], dt.bfloat16, tag="ps", name=f"tp{j}")
            for i in range(8):
                g = j * 8 + i
                csl = slice(i * 128, (i + 1) * 128)
                nc.tensor.transpose(
                    out=tp[0:16, csl],
                    in_=probs_hl[:, g, :],
                    identity=identb[:],
                )
            sl = slice(j * 1024, (j + 1) * 1024)
            nc.vector.tensor_copy(pT_stack[0:16, sl], tp[0:16, :])
            # duplicate ph into rows 16:24 per chunk (HWDGE SBUF-to-SBUF;
            # the SWDGE/gpsimd path faults the device on HW)
            nc.sync.dma_start(pT_stack[16:24, sl], pT_stack[0:8, sl])

        # qT = comp^T @ pTq exact fp32, then bf16 hi/lo into the stacked
        # lhsT layout: [0:8]=qh, [32:40]=qh, [64:72]=ql; gap rows zeroed.
        qT_stack = sb.tile([KS, TQ], dt.bfloat16)
        qh_tmp = sb.tile([C, TQ], dt.bfloat16)
        ql_tmp = sb.tile([C, TQ], dt.bfloat16)
        for i in range(TQ // NTILE):
            qp = ps.tile([C, NTILE], dt.float32, tag="ps", name=f"qp{i}")
            nc.tensor.matmul(
                out=qp[:],
                lhsT=compv[:],
                rhs=pTq[:, i * NTILE : (i + 1) * NTILE],
                start=True,
                stop=True,
            )
            qsl = slice(i * NTILE, (i + 1) * NTILE)
            nc.vector.tensor_copy(qh_tmp[:, qsl], qp[:])
            nc.vector.tensor_sub(ql_tmp[:, qsl], qp[:], qh_tmp[:, qsl])
            nc.vector.tensor_copy(qT_stack[0:8, qsl], qh_tmp[:, qsl])
            nc.sync.dma_start(qT_stack[8:16, qsl], qh_tmp[:, qsl])
            nc.sync.dma_start(qT_stack[16:24, qsl], ql_tmp[:, qsl])

        # main: compat tile = qT_m^T @ pT_n, tanh(x/2) over matmul PAIRS
        # (half the ACT dispatches), variable-size contiguous stripe DMAs
        m = 0
        for nm in STRIPE_M:
            stripe = stripes.tile([128, nm * T], dt.float32, name="stripe")
            for s_ in range(nm):
                msl = slice((m + s_) * 128, (m + s_ + 1) * 128)
                for n2 in range(T // (2 * NTILE)):
                    po = ps.tile(
                        [128, 2 * NTILE], dt.float32, tag="ps",
                        name=f"po{m + s_}_{n2}",
                    )
                    for h in range(2):
                        n = 2 * n2 + h
                        nc.tensor.matmul(
                            out=po[:, h * NTILE : (h + 1) * NTILE],
                            lhsT=qT_stack[:, msl],
                            rhs=pT_stack[:, n * NTILE : (n + 1) * NTILE],
                            start=True,
                            stop=True,
                        )
                    nc.scalar.activation(
                        stripe[
                            :,
                            s_ * T + n2 * 2 * NTILE : s_ * T + (n2 + 1) * 2 * NTILE,
                        ],
                        po[:],
                        mybir.ActivationFunctionType.Tanh,
                        scale=0.5,
                    )
            dst = out[m * 128 : (m + nm) * 128, :]
            if nm > 1:
                dst = dst.rearrange("(s p) c -> p (s c)", s=nm)
            nc.sync.dma_start(dst, stripe[:])
            m += nm

    return nc


def _get_nc():
    if "nc" not in _CACHE:
        nc = _build()
        # Bacc defers register allocation to finalize(); the bass2jax SPMD
        # path serializes nc.m as-is, so finalize before handing it over.
        nc.finalize()
        _CACHE["nc"] = nc
    return _CACHE["nc"]


def _make_idx(tok_b: np.ndarray, t0: int):
    """int16 index pair for the two-slab gather, wrapped for dma_gather:
    logical token j (= g*128+p) sits at idx[j%16, j//16], replicated to
    all 8 GPSIMD-core partition groups. Key order is query-half-first."""
    oth = TQ - t0
    tokens = np.concatenate(
        [tok_b[t0 : t0 + TQ], tok_b[oth : oth + TQ]]
    ).astype(np.int64)
    w = tokens.reshape(G * 8, 16).T  # w[p, s] = tokens[s*16+p]
    i1 = np.where(w < HALF, w + 1, 0).astype(np.int16)
    i2 = np.where(w >= HALF, w - HALF + 1, 0).astype(np.int16)
    return np.tile(i1, (8, 1)), np.tile(i2, (8, 1))


def _make_slabs(emb: np.ndarray) -> np.ndarray:
    slabs = np.zeros((2, HALF + 1, PAD), np.float32)
    slabs[0, 1 : HALF + 1, :C] = emb[0:HALF]
    slabs[1, 1 : VOCAB - HALF + 1, :C] = emb[HALF:]
    return slabs


def _make_in_maps(tok, emb, comp):
    slabs = _make_slabs(emb)
    in_maps = []
    for k in range(NCORES):
        b, t0 = k // 2, (k % 2) * TQ
        i1, i2 = _make_idx(tok[b], t0)
        in_maps.append({"slabs": slabs, "comp": comp, "idx1": i1, "idx2": i2})
    return in_maps


def kernel(token_ids, dim_embedding, compatibility):
    global LAST_RESULT
    from concourse.bass_utils import run_bass_kernel_spmd

    tok = np.asarray(token_ids)
    emb = np.ascontiguousarray(np.asarray(dim_embedding, dtype=np.float32))
    comp = np.ascontiguousarray(np.asarray(compatibility, dtype=np.float32))
    assert tok.shape == (B, T) and emb.shape == (VOCAB, C) and comp.shape == (C, C)

    nc = _get_nc()
    in_maps = _make_in_maps(tok, emb, comp)

    res = run_bass_kernel_spmd(nc, in_maps, list(range(NCORES)))
    LAST_RESULT = res

    full = np.empty((B, T, T), dtype=np.float32)
    for k in range(NCORES):
        b, t0 = k // 2, (k % 2) * TQ
        oth = TQ - t0
        o = res.results[k]["out"]
        full[b, t0 : t0 + TQ, t0 : t0 + TQ] = o[:, :TQ]
        full[b, t0 : t0 + TQ, oth : oth + TQ] = o[:, TQ:]
    return full
